# revision 1
# baseline (speedup 1.0000x reference)
"""Trainium2 Bass kernel for nn_C_BatchNorm (complex batch-norm, training).

v2: bf16 device I/O + component-deinterleaved layout + big DMAs.

Problem: z [B=32, C=128, H=64, W=64, 2] fp32.  Per position n=(c,h,w):
2x2 covariance over batch, closed-form inverse sqrt, whiten, gamma/beta.
Error gate is 2e-2; bf16 end-to-end measures ~4e-3, so the device works in
bf16 (fp32 PSUM accumulation for stats, fp32 phase-2 coefficient math) and
the device I/O is bf16: 8 MiB in + 8 MiB out per core instead of 32 MiB.

Sharding: C split across 8 cores (16 channels each).  Host packs the core
shard as zin[128, 32768] bf16 where row r = 32*j + b (j = position group,
b = batch) and col block t in [0,32) holds [z0_t(512) | z1_t(512)] =
component-deinterleaved positions 512t..512t+512 of each group.  Output is
the same packing; host upcasts to fp32 + unpacks.

Per-core algorithm:
  Phase 1: z loaded in 4x 2 MiB DMAs.  Per tile t: zz = ACT square (bf16),
    zx = DVE stride-1 bf16 2x mult; 5 bf16 selector matmuls reduce over the
    32 batch partitions of each group.  Stats for 8-tile block g land in
    PSUM partition strip 32g..32g+31 via PE column tiling
    (tile_position=(0,32g)), so staging to SBUF is a plain engine copy --
    no cross-partition repack DMAs.
  Phase 2 (once, fp32 planes [128,512]): closed-form 2x2 inverse sqrt,
    gamma fold, bias fold -> bf16 planes AB0=[A00|A01], AB1=[A10|A11],
    RB=[R0|R1]   (out0 = A00 z0 + A01 z1 + R0, out1 = A10 z0 + A11 z1 + R1).
  Phase 3 (per tile): indicator matmuls broadcast the tile's 4 coefficient
    rows to 128 partitions (PSUM), PSUM->SBUF bf16 copies, 8 stride-1 bf16
    2x TT ops apply the transform into an 8-tile output buffer, stored with
    4x 2 MiB DMAs.
"""

import numpy as np
import ml_dtypes

import concourse.bass as bass
import concourse.bacc as bacc
import concourse.tile as tile
from concourse import mybir
from concourse.bass_utils import run_bass_kernel_spmd

f32 = mybir.dt.float32
bf16 = mybir.dt.bfloat16
AF = mybir.ActivationFunctionType
OP = mybir.AluOpType
BF = ml_dtypes.bfloat16

# ---- problem geometry (hardcoded) ----
B, C, H, W = 32, 128, 64, 64
NCORES = 8
C_PER = C // NCORES                  # 16 channels per core
NPOS = C_PER * H * W                 # 65536 positions per core
J = 4                                # position groups (rows 32j+b)
NT = 32                              # tiles per core
FP = 512                             # positions per group per tile
TCOLS = 2 * FP                       # 1024 cols per tile block [z0|z1]
ZCOLS = NT * TCOLS                   # 32768 cols total
NB = 512                             # matmul free-dim chunk (one PSUM bank)
LCH = 8                              # tiles per input-load DMA (4 loads)
SCH = 8                              # tiles per output-store DMA (4 stores)
P3_SPLIT = "6-2"                     # DVE/gpsimd TT split in phase 3
PSR_BUFS = 2                         # double-buffer the R-broadcast PSUM
PHASE_UPTO = "full"                  # 'p1' | 'p2' | 'full' (leg measurement)


def _host_constants():
    # selector weights for phase-1 stats packing: 8 variants [128, 32],
    # variant i: sel[p, w] = 1 iff w == 4*i + p//32
    sel8 = np.zeros((128, 8, 32), dtype=BF)
    for i in range(8):
        for p in range(128):
            sel8[p, i, 4 * i + p // 32] = 1.0
    sel8 = sel8.reshape(128, 8 * 32)
    # indicator for phase-3 broadcast: 8 variants [32, 128] tiled to 128 rows:
    # ind[p, 128*i + q] = 1 iff (p % 32) == 4*i + q//32
    ind = np.zeros((128, 8, 128), dtype=BF)
    for i in range(8):
        for p in range(128):
            for jj in range(4):
                if p % 32 == 4 * i + jj:
                    ind[p, i, 32 * jj:32 * (jj + 1)] = 1.0
    ind = ind.reshape(128, 8 * 128)
    return sel8, ind


def _pack_core(zsh):
    """zsh [B, NPOS, 2] fp32 -> [128, ZCOLS] bf16 device layout."""
    a = zsh.reshape(B, J, NT, FP, 2)          # [b, j, t, k, i]
    a = a.transpose(1, 0, 2, 4, 3)            # [j, b, t, i, k]
    return np.ascontiguousarray(a.reshape(128, ZCOLS).astype(BF))


def _unpack_core(o):
    """[128, ZCOLS] bf16 device layout -> [B, NPOS, 2] fp32."""
    a = np.asarray(o).reshape(J, B, NT, 2, FP).astype(np.float32)
    a = a.transpose(1, 0, 2, 4, 3)            # [b, j, t, k, i]
    return a.reshape(B, NPOS, 2)


def build_module(reps=1):
    nc = bacc.Bacc("TRN2", target_bir_lowering=False, debug=False,
                   detect_race_conditions=False)
    z_d = nc.dram_tensor("z", [128, ZCOLS], bf16, kind="ExternalInput").ap()
    gamma_d = nc.dram_tensor("gamma", [2, 2], f32, kind="ExternalInput").ap()
    beta_d = nc.dram_tensor("beta", [2], f32, kind="ExternalInput").ap()
    sel8_d = nc.dram_tensor("sel8", [128, 8 * 32], bf16, kind="ExternalInput").ap()
    ind_d = nc.dram_tensor("ind", [128, 8 * 128], bf16, kind="ExternalInput").ap()
    out_d = nc.dram_tensor("out", [128, ZCOLS], bf16, kind="ExternalOutput").ap()

    irB = 1.0 / np.sqrt(np.float32(B))      # 1/sqrt(B)

    with tile.TileContext(nc) as tc:
        with (
            tc.tile_pool(name="consts", bufs=1) as consts,
            tc.tile_pool(name="zres", bufs=1) as zres,
            tc.tile_pool(name="stats", bufs=1) as stats,
            tc.tile_pool(name="ph2", bufs=1) as ph2,
            tc.tile_pool(name="coef", bufs=1) as coef,
            tc.tile_pool(name="work", bufs=2) as work,
            tc.tile_pool(name="owork", bufs=2) as owork,
        ):
            # ---------- constants ----------
            sel_sb = consts.tile([128, 8 * 32], bf16)
            nc.sync.dma_start(out=sel_sb[:], in_=sel8_d)
            ind_sb = consts.tile([128, 8 * 128], bf16)
            nc.sync.dma_start(out=ind_sb[:], in_=ind_d)

            gcols = consts.tile([128, 6], f32)   # g00 g01 g10 g11 b0 b1
            for k in range(4):
                nc.gpsimd.dma_start(
                    out=gcols[:, k:k + 1],
                    in_=bass.AP(tensor=gamma_d.tensor, offset=k,
                                ap=[[0, 128], [1, 1]]))
            for k in range(2):
                nc.gpsimd.dma_start(
                    out=gcols[:, 4 + k:5 + k],
                    in_=bass.AP(tensor=beta_d.tensor, offset=k,
                                ap=[[0, 128], [1, 1]]))
            g00c, g01c = gcols[:, 0:1], gcols[:, 1:2]
            g10c, g11c = gcols[:, 2:3], gcols[:, 3:4]
            b0c, b1c = gcols[:, 4:5], gcols[:, 5:6]

            # resident z for the whole core (bf16)
            z_all = zres.tile([128, ZCOLS], bf16)

            # fp32 stats planes
            S01 = stats.tile([128, TCOLS], f32)      # [S0 | S1]
            Q01 = stats.tile([128, TCOLS], f32)      # [Q0 | Q1]
            Xp = stats.tile([128, FP], f32)          # X

            def _pipeline(rep):
                # ---------- phase 1: moments, packed into PSUM ----------
                with tc.tile_pool(name="psum1", bufs=1, space="PSUM") as psum1:
                    ps_S = psum1.tile([128, TCOLS], f32)
                    ps_Q = psum1.tile([128, TCOLS], f32)
                    ps_X = psum1.tile([128, FP], f32)
                    for t in range(NT):
                        g, i = divmod(t, 8)
                        if t % LCH == 0:
                            cs = slice(t * TCOLS, (t + LCH) * TCOLS)
                            nc.sync.dma_start(out=z_all[:, cs], in_=z_d[:, cs])
                        zv = z_all[:, t * TCOLS:(t + 1) * TCOLS]
                        z0v = z_all[:, t * TCOLS:t * TCOLS + FP]
                        z1v = z_all[:, t * TCOLS + FP:(t + 1) * TCOLS]
                        zz = work.tile([128, TCOLS], bf16, tag="zz")
                        nc.scalar.square(zz[:], zv)
                        zx = work.tile([128, FP], bf16, tag="zx")
                        nc.vector.tensor_tensor(zx[:], z0v, z1v, OP.mult)

                        lhs = sel_sb[:, 32 * i:32 * (i + 1)]
                        st = i == 0
                        sp = i == 7
                        rows = slice(32 * g, 32 * (g + 1))
                        tp = (0, 32 * g)
                        for h in range(2):
                            hs = slice(h * NB, (h + 1) * NB)
                            nc.tensor.matmul(ps_S[rows, hs], lhs,
                                             zv[:, hs],
                                             start=st, stop=sp,
                                             tile_position=tp,
                                             skip_group_check=True)
                            nc.tensor.matmul(ps_Q[rows, hs], lhs,
                                             zz[:, hs],
                                             start=st, stop=sp,
                                             tile_position=tp,
                                             skip_group_check=True)
                        nc.tensor.matmul(ps_X[rows, :], lhs,
                                         zx[:],
                                         start=st, stop=sp,
                                         tile_position=tp,
                                         skip_group_check=True)

                        if i == 7:
                            # stage this block's strip (same partitions, no DMA)
                            nc.scalar.copy(S01[rows, :], ps_S[rows, :])
                            nc.vector.tensor_copy(Q01[rows, :], ps_Q[rows, :])
                            nc.vector.tensor_copy(Xp[rows, :], ps_X[rows, :])

                if PHASE_UPTO == "p1":
                    return
                # ---------- phase 2: closed-form 2x2 inverse sqrt ----------
                S0, S1 = S01[:, 0:FP], S01[:, FP:TCOLS]
                Q0, Q1 = Q01[:, 0:FP], Q01[:, FP:TCOLS]

                P00 = ph2.tile([128, FP], f32)
                P11 = ph2.tile([128, FP], f32)
                P01 = ph2.tile([128, FP], f32)
                d_ = ph2.tile([128, FP], f32)
                s_ = ph2.tile([128, FP], f32)
                u_ = ph2.tile([128, FP], f32)
                r_ = ph2.tile([128, FP], f32)
                # aliases onto dead scratch (lifetimes are disjoint):
                q2 = P00      # P00 dead after Q0 -= P00
                rsc = P01     # P01 dead after Xp -= P01 (recip scratch)
                w00 = P11     # P11 dead after Q1 -= P11
                w11 = d_      # d_ dead after s_ = sqrt(d_)
                tq = q2       # q2 dead after d_ -= q2   (same mem as P00)

                # P terms
                nc.scalar.activation(P00[:], S0, AF.Square, scale=float(irB))
                nc.scalar.activation(P11[:], S1, AF.Square, scale=float(irB))
                nc.vector.scalar_tensor_tensor(P01[:], S0, float(1.0 / B),
                                               S1, OP.mult, OP.mult)
                # sigma' (in place into Q0/Q1/Xp)
                nc.vector.tensor_tensor(Q0, Q0, P00[:], OP.subtract)
                nc.vector.tensor_tensor(Q1, Q1, P11[:], OP.subtract)
                nc.vector.tensor_tensor(Xp[:], Xp[:], P01[:], OP.subtract)
                # det' = Q0*Q1 - Xp^2
                nc.vector.tensor_tensor(d_[:], Q0, Q1, OP.mult)
                nc.scalar.square(q2[:], Xp[:])
                nc.vector.tensor_tensor(d_[:], d_[:], q2[:], OP.subtract)
                nc.scalar.activation(s_[:], d_[:], AF.Sqrt)
                # u' = Q0 + Q1 + 2 s'   (u_ holds trace, then u')
                nc.vector.tensor_tensor(u_[:], Q0, Q1, OP.add)
                nc.vector.scalar_tensor_tensor(u_[:], s_[:], 2.0, u_[:],
                                               OP.mult, OP.add)
                # tq = sqrt((B-1) u') ; r = 1/tq
                nc.scalar.activation(tq[:], u_[:], AF.Sqrt, scale=float(B - 1))
                nc.vector.reciprocal_approx_accurate(r_[:], tq[:], rsc[:])
                # w00 = (Q0 + s') r ; w11 = (Q1 + s') r ; w01 = Xp r (in Xp)
                nc.vector.tensor_tensor(w00[:], Q0, s_[:], OP.add)
                nc.vector.tensor_tensor(w00[:], w00[:], r_[:], OP.mult)
                nc.vector.tensor_tensor(w11[:], Q1, s_[:], OP.add)
                nc.vector.tensor_tensor(w11[:], w11[:], r_[:], OP.mult)
                nc.vector.tensor_tensor(Xp[:], Xp[:], r_[:], OP.mult)
                w01 = Xp[:]

                # A coefficients -> bf16 planes + fp32 staging for R
                AB0 = coef.tile([128, TCOLS], bf16)   # [A00 | A01]
                AB1 = coef.tile([128, TCOLS], bf16)   # [A10 | A11]
                RB = coef.tile([128, TCOLS], bf16)    # [R0 | R1]
                A00f = ph2.tile([128, FP], f32)
                A01f = ph2.tile([128, FP], f32)
                A10f = ph2.tile([128, FP], f32)
                A11f = ph2.tile([128, FP], f32)
                t0 = u_       # u_ dead after tq
                t1 = s_       # s_ dead after w00/w11

                nc.vector.tensor_scalar(t0[:], w01, g01c, None, OP.mult)
                nc.vector.scalar_tensor_tensor(A00f[:], w00[:], g00c, t0[:],
                                               OP.mult, OP.add)
                nc.vector.tensor_scalar(t1[:], w01, g11c, None, OP.mult)
                nc.vector.scalar_tensor_tensor(A10f[:], w00[:], g10c, t1[:],
                                               OP.mult, OP.add)
                nc.vector.tensor_scalar(t0[:], w11[:], g01c, None, OP.mult)
                nc.vector.scalar_tensor_tensor(A01f[:], w01, g00c, t0[:],
                                               OP.mult, OP.add)
                nc.vector.tensor_scalar(t1[:], w11[:], g11c, None, OP.mult)
                nc.vector.scalar_tensor_tensor(A11f[:], w01, g10c, t1[:],
                                               OP.mult, OP.add)
                nc.scalar.copy(AB0[:, 0:FP], A00f[:])
                nc.scalar.copy(AB0[:, FP:TCOLS], A01f[:])
                nc.scalar.copy(AB1[:, 0:FP], A10f[:])
                nc.scalar.copy(AB1[:, FP:TCOLS], A11f[:])

                # R = beta - (A . S)/B    (c0/c1 reuse dead scratch)
                c0, c1 = w00, w11
                nc.vector.scalar_tensor_tensor(c0[:], S0, float(1.0 / B),
                                               A00f[:], OP.mult, OP.mult)
                nc.vector.scalar_tensor_tensor(c1[:], S1, float(1.0 / B),
                                               A01f[:], OP.mult, OP.mult)
                nc.vector.tensor_tensor(c0[:], c0[:], c1[:], OP.add)
                nc.vector.tensor_scalar(RB[:, 0:FP], c0[:], -1.0, b0c,
                                        OP.mult, OP.add)
                nc.vector.scalar_tensor_tensor(c0[:], S0, float(1.0 / B),
                                               A10f[:], OP.mult, OP.mult)
                nc.vector.scalar_tensor_tensor(c1[:], S1, float(1.0 / B),
                                               A11f[:], OP.mult, OP.mult)
                nc.vector.tensor_tensor(c0[:], c0[:], c1[:], OP.add)
                nc.vector.tensor_scalar(RB[:, FP:TCOLS], c0[:], -1.0, b1c,
                                        OP.mult, OP.add)

                if PHASE_UPTO == "p2":
                    return
                # ---------- phase 3: broadcast + apply ----------
                with tc.tile_pool(name="psum3", bufs=1, space="PSUM") as psum3:
                    ob = None
                    for t in range(NT):
                        g, i = divmod(t, 8)
                        rows = slice(32 * g, 32 * (g + 1))
                        lhs_b = ind_sb[rows, 128 * i:128 * (i + 1)]
                        ps_A0 = psum3.tile([128, TCOLS], f32, tag="psA0")
                        ps_A1 = psum3.tile([128, TCOLS], f32, tag="psA1")
                        ps_R = psum3.tile([128, TCOLS], f32, tag="psR",
                                          bufs=PSR_BUFS)
                        for h in range(2):
                            hs = slice(h * NB, (h + 1) * NB)
                            nc.tensor.matmul(ps_A0[:, hs], lhs_b,
                                             AB0[rows, hs],
                                             start=True, stop=True,
                                             tile_position=(32 * g, 0),
                                             skip_group_check=True)
                            nc.tensor.matmul(ps_A1[:, hs], lhs_b,
                                             AB1[rows, hs],
                                             start=True, stop=True,
                                             tile_position=(32 * g, 0),
                                             skip_group_check=True)
                            nc.tensor.matmul(ps_R[:, hs], lhs_b,
                                             RB[rows, hs],
                                             start=True, stop=True,
                                             tile_position=(32 * g, 0),
                                             skip_group_check=True)
                        cA0 = work.tile([128, TCOLS], bf16, tag="cA0")
                        cA1 = work.tile([128, TCOLS], bf16, tag="cA1")
                        cR = work.tile([128, TCOLS], bf16, tag="cR")
                        nc.scalar.copy(cA0[:], ps_A0)
                        nc.scalar.copy(cA1[:], ps_A1)
                        nc.vector.tensor_copy(cR[:], ps_R)

                        z0v = z_all[:, t * TCOLS:t * TCOLS + FP]
                        z1v = z_all[:, t * TCOLS + FP:(t + 1) * TCOLS]
                        if t % SCH == 0:
                            ob = owork.tile([128, SCH * TCOLS], bf16, tag="ob")
                        oo = (t % SCH) * TCOLS
                        o0 = ob[:, oo:oo + FP]
                        o1 = ob[:, oo + FP:oo + TCOLS]

                        m0 = work.tile([128, TCOLS], bf16, tag="m0")
                        m1 = work.tile([128, TCOLS], bf16, tag="m1")
                        if P3_SPLIT == "6-2":
                            e_m1a, e_m1b = nc.vector, nc.vector
                        else:
                            e_m1a, e_m1b = nc.gpsimd, nc.gpsimd
                        nc.vector.tensor_tensor(m0[:, 0:FP], z0v,
                                                cA0[:, 0:FP], OP.mult)
                        nc.vector.tensor_tensor(m0[:, FP:TCOLS], z1v,
                                                cA0[:, FP:TCOLS], OP.mult)
                        e_m1a.tensor_tensor(m1[:, 0:FP], z0v,
                                            cA1[:, 0:FP], OP.mult)
                        e_m1b.tensor_tensor(m1[:, FP:TCOLS], z1v,
                                            cA1[:, FP:TCOLS], OP.mult)
                        nc.vector.tensor_tensor(m0[:, 0:FP], m0[:, 0:FP],
                                                m0[:, FP:TCOLS], OP.add)
                        nc.gpsimd.tensor_tensor(m1[:, 0:FP], m1[:, 0:FP],
                                                m1[:, FP:TCOLS], OP.add)
                        nc.vector.tensor_tensor(o0, m0[:, 0:FP],
                                                cR[:, 0:FP], OP.add)
                        nc.gpsimd.tensor_tensor(o1, m1[:, 0:FP],
                                                cR[:, FP:TCOLS], OP.add)
                        if t % SCH == SCH - 1:
                            cs = slice((t - SCH + 1) * TCOLS, (t + 1) * TCOLS)
                            nc.scalar.dma_start(out=out_d[:, cs], in_=ob[:])

            for _rep in range(reps):
                _pipeline(_rep)

    nc.compile()
    return nc


_NC = {}


def _get_module(reps=1):
    if reps not in _NC:
        _NC[reps] = build_module(reps)
    return _NC[reps]


def _make_in_maps(z, gamma, beta):
    z = np.ascontiguousarray(z, dtype=np.float32)
    zr = z.reshape(B, C, H * W * 2)
    sel8, ind = _host_constants()
    gamma = np.ascontiguousarray(gamma, np.float32)
    beta = np.ascontiguousarray(beta, np.float32)
    in_maps = []
    for c in range(NCORES):
        zsh = zr[:, c * C_PER:(c + 1) * C_PER].reshape(B, NPOS, 2)
        in_maps.append({"z": _pack_core(zsh), "gamma": gamma, "beta": beta,
                        "sel8": sel8, "ind": ind})
    return in_maps


def _gather_out(results):
    out = np.empty((B, C, H * W * 2), dtype=np.float32)
    for c in range(NCORES):
        o = _unpack_core(results[c]["out"])
        out[:, c * C_PER:(c + 1) * C_PER] = o.reshape(B, C_PER, H * W * 2)
    return out.reshape(B, C, H, W, 2)


def kernel(z, gamma, beta):
    in_maps = _make_in_maps(z, gamma, beta)
    m1 = _get_module(1)
    runner = _get_runner(("m", id(m1)), m1, NCORES)
    results = _run_module(runner, in_maps)
    return _gather_out(results)


# ---------------- runner / bench infra (same as baseline) ----------------

def _make_runner(nc, n_cores):
    import jax
    from jax.sharding import Mesh, PartitionSpec
    from jax.experimental.shard_map import shard_map
    from concourse import bass2jax
    from concourse.bass2jax import _bass_exec_p, install_neuronx_cc_hook
    from concourse import mybir as _mb

    install_neuronx_cc_hook()
    partition_name = (nc.partition_id_tensor.name
                      if nc.partition_id_tensor else None)
    in_names, out_names, out_avals, zero_outs = [], [], [], []
    for alloc in nc.m.functions[0].allocations:
        if not isinstance(alloc, _mb.MemoryLocationSet):
            continue
        name = alloc.memorylocations[0].name
        if alloc.kind == "ExternalInput":
            if name != partition_name:
                in_names.append(name)
        elif alloc.kind == "ExternalOutput":
            shape = tuple(alloc.tensor_shape)
            dtype = _mb.dt.np(alloc.dtype)
            out_names.append(name)
            out_avals.append(jax.core.ShapedArray(shape, dtype))
            zero_outs.append(np.zeros(shape, dtype))
    n_params = len(in_names)
    n_outs = len(out_avals)
    all_in_names = in_names + out_names
    if partition_name is not None:
        all_in_names.append(partition_name)

    def _body(*args):
        operands = list(args)
        if partition_name is not None:
            operands.append(bass2jax.partition_id_tensor())
        outs = _bass_exec_p.bind(
            *operands,
            out_avals=tuple(out_avals),
            in_names=tuple(all_in_names),
            out_names=tuple(out_names),
            lowering_input_output_aliases=(),
            sim_require_finite=True,
            sim_require_nnan=True,
            nc=nc,
        )
        return tuple(outs)

    devices = jax.devices()[:n_cores]
    mesh = Mesh(np.asarray(devices), ("core",))
    donate = tuple(range(n_params, n_params + n_outs))
    sharded = jax.jit(
        shard_map(_body, mesh=mesh,
                  in_specs=(PartitionSpec("core"),) * (n_params + n_outs),
                  out_specs=(PartitionSpec("core"),) * n_outs,
                  check_rep=False),
        donate_argnums=donate, keep_unused=True,
    )
    from jax.sharding import NamedSharding
    shard0 = NamedSharding(mesh, PartitionSpec("core"))
    return {
        "sharded": sharded, "shard0": shard0, "in_names": in_names,
        "out_names": out_names, "out_avals": out_avals,
        "zero_outs": zero_outs, "n_cores": n_cores,
    }


_RUNNERS = {}


def _get_runner(key, nc, n_cores):
    if key not in _RUNNERS:
        _RUNNERS[key] = _make_runner(nc, n_cores)
    return _RUNNERS[key]


def _run_module(runner, in_maps):
    import jax
    n_cores = runner["n_cores"]
    concat_in = [
        jax.device_put(
            np.concatenate([np.asarray(m[name]) for m in in_maps], axis=0),
            runner["shard0"])
        for name in runner["in_names"]
    ]
    zeros = [
        jax.device_put(
            np.zeros((n_cores * z.shape[0], *z.shape[1:]), z.dtype),
            runner["shard0"])
        for z in runner["zero_outs"]
    ]
    outs = runner["sharded"](*concat_in, *zeros)
    jax.block_until_ready(outs)
    return [
        {name: np.asarray(outs[i]).reshape(
            n_cores, *runner["out_avals"][i].shape)[c]
         for i, name in enumerate(runner["out_names"])}
        for c in range(n_cores)
    ]


def bench_pair(ncs, in_maps_a, in_maps_b, iters=8, rounds=4):
    import time as _time
    import jax
    runners = [_get_runner(("m", id(ncs[0])), ncs[0], len(in_maps_a)),
               _get_runner(("m", id(ncs[1])), ncs[1], len(in_maps_b))]
    sides = []
    for runner, im in ((runners[0], in_maps_a), (runners[1], in_maps_b)):
        concat_in = [
            jax.device_put(
                np.concatenate([np.asarray(m[name]) for m in im], axis=0),
                runner["shard0"])
            for name in runner["in_names"]
        ]
        n_cores = runner["n_cores"]
        zsets = []
        for _ in range(iters * rounds + 1):
            zsets.append([
                jax.device_put(
                    np.zeros((n_cores * z.shape[0], *z.shape[1:]), z.dtype),
                    runner["shard0"])
                for z in runner["zero_outs"]
            ])
        sides.append((runner, concat_in, zsets))
        out = runner["sharded"](*concat_in, *zsets[0])
        jax.block_until_ready(out)
    ta, tb = [], []
    k = [0, 0]
    for r in range(rounds):
        for side, rec in ((0, ta), (1, tb)):
            runner, concat_in, zsets = sides[side]
            t0 = _time.perf_counter()
            last = None
            for _ in range(iters):
                k[side] += 1
                last = runner["sharded"](*concat_in, *zsets[k[side]])
            jax.block_until_ready(last)
            rec.append((_time.perf_counter() - t0) / iters * 1e9)
    return ta, tb


def bench(z, gamma, beta, iters=10, reps=17, with_memcpy=False):
    in_maps = _make_in_maps(z, gamma, beta)
    ta, tb = bench_pair((_get_module(1), _get_module(reps)),
                        in_maps, in_maps, iters=iters, rounds=10)
    # Per-round slope cancels dispatch overhead; median over many rounds
    # rides out the large ambient noise of the shared tunnel.
    slopes = sorted((b - a) / (reps - 1) for a, b in zip(ta, tb))
    ns = slopes[len(slopes) // 2]
    m1 = _get_module(1)
    runner = _get_runner(("m", id(m1)), m1, NCORES)
    results = _run_module(runner, in_maps)
    t1_ns, tR_ns = min(ta), min(tb)
    return _gather_out(results), ns, (t1_ns, tR_ns)


def run_traced(z, gamma, beta):
    in_maps = _make_in_maps(z, gamma, beta)
    nc = _get_module()
    res = run_bass_kernel_spmd(nc, in_maps, core_ids=list(range(NCORES)),
                               trace=True)
    return _gather_out(res.results), res.exec_time_ns, res

